# revision 4
# baseline (speedup 1.0000x reference)
"""GCNII (nn_GCNII_17626545783193) Bass/Trainium2 kernel, 8 NeuronCores.

Strategy (target-node sharding, feature-major compute, bf16 aggregation):
  - Nodes sharded 12500/core (padded to 12544 = 98*128). Edges partitioned
    by target core; self-loops included as real edges.
  - Hidden states hs = dinv*h stored in bf16 with DUPLICATED halves:
    hs_full is [100352, 128] bf16 where row n = [h_n | h_n]. dma_gather
    reads 256B rows (the gather stride/elem floor is 256B); the matmul
    stationary always slices columns 0:64.
  - gcn_norm factored: norm[e] = dinv[src]*dinv[dst]; dinv[src] folded into
    hs, 0.9*dinv[dst] folded into the one-hot scatter matrix (dcol).
  - Per layer, per core: dma_gather of in-edge source rows from hs_full;
    DVE builds bf16 one-hot [128edge, 128target] tiles (iota==tloc)*dcol
    at 4x mode; PE computes agg^T = msgs^T @ onehot (bf16, 1 cyc/row)
    accumulated in PSUM [64, 512]; support = agg + 0.1*h0^T (DVE); GCNII
    dense update via ONE matmul with host-fused W' = beta*Wl + (1-beta)*I;
    ACT relu; PE transpose back to node-major; scale by dinv; the shard
    row is DMAed into both 64-col halves of hs_shard; AllGather.
"""
import sys
sys.path.insert(0, "/opt/trn_rl_repo")

import math
import numpy as np

# ---- problem constants (hardcoded per spec) ----
N_NODES = 100000
N_FEATS = 128
HIDDEN = 64
N_CLASSES = 40
N_LAYERS = 8
ALPHA = 0.1
LAMDA = 0.5
NCORES = 8

NPC = N_NODES // NCORES            # 12500 real nodes per core
TPC = (NPC + 127) // 128           # 98 target tiles per core
NPC_PAD = TPC * 128                # 12544
N_PAD = NPC_PAD * NCORES           # 100352
GROUP_TT = 8                       # target tiles per gather group
SUB_TT = 4                         # target tiles per psum tile (512 cols)
SRC_CHUNK = 32768                  # int16 index range per gather source chunk
NCHUNKS = (N_PAD + SRC_CHUNK - 1) // SRC_CHUNK  # 4
CALL_MAX = 8192                    # max indices per dma_gather call

TRACE = False          # test.py sets this for profiling
_LAST_RESULT = {}      # test.py reads exec_time from here


def _preprocess(edge_index):
    """Build the uniform (cross-core identical) edge stream structure and the
    per-core data arrays.  Slots are keyed (target_tile, src_chunk, parity):
    parity = source node id & 1 selects which half of the gathered 256B
    pair-row holds the source features."""
    row = np.asarray(edge_index[0], dtype=np.int64)
    col = np.asarray(edge_index[1], dtype=np.int64)
    deg = np.bincount(col, minlength=N_NODES).astype(np.float32) + 1.0
    dinv = (1.0 / np.sqrt(deg)).astype(np.float32)

    loops = np.arange(N_NODES, dtype=np.int64)
    r = np.concatenate([row, loops])
    cl = np.concatenate([col, loops])
    core = cl // NPC
    lcol = cl - core * NPC                       # 0..12499 local target
    tt = lcol >> 7                               # target tile 0..97
    gp = (r // NPC) * NPC_PAD + (r % NPC)        # padded global source id
    pr = gp >> 1                                 # pair row 0..50175
    pi = gp & 1                                  # parity within pair row
    k = pr // SRC_CHUNK                          # source chunk 0..1

    ngroups = (TPC + GROUP_TT - 1) // GROUP_TT

    slot = ((core * TPC + tt) * NCHUNKS + k) * 2 + pi
    counts = np.bincount(slot, minlength=NCORES * TPC * NCHUNKS * 2).reshape(
        NCORES, TPC, NCHUNKS, 2
    )
    maxc = counts.max(axis=0)                    # [TPC, NCHUNKS, 2]
    slot_chunks = -(-maxc // 128)                # ceil; 0 allowed
    slot_len = slot_chunks * 128

    slot_off = np.zeros((TPC, NCHUNKS, 2), np.int64)
    off = 0
    chunk_tt = []
    chunk_par = []
    calls_by_group = [[] for _ in range(ngroups)]
    for g in range(ngroups):
        tts = range(g * GROUP_TT, min((g + 1) * GROUP_TT, TPC))
        for kk in range(NCHUNKS):
            run_start = off
            for pp in range(2):
                for t in tts:
                    slot_off[t, kk, pp] = off
                    off += slot_len[t, kk, pp]
                    chunk_tt += [t] * slot_chunks[t, kk, pp]
                    chunk_par += [pp] * slot_chunks[t, kk, pp]
            s = run_start
            while s < off:
                n = min(CALL_MAX, off - s)
                calls_by_group[g].append((kk, int(s), int(n)))
                s += n
    e_pad = off
    nchk = e_pad // 128
    chunk_tt = np.asarray(chunk_tt, np.int64)
    chunk_par = np.asarray(chunk_par, np.int64)
    # start/stop are per PSUM tile (= subgroup of SUB_TT ttiles): exactly one
    # start=True (chronologically first matmul into the bank) and one
    # stop=True (last). A second start into a live bank wipes it (bank-wide
    # pending-zero), so interleaved per-ttile groups are NOT allowed.
    sub_id = chunk_tt // SUB_TT
    first = np.zeros(nchk, bool)
    last = np.zeros(nchk, bool)
    for sg in np.unique(sub_id):
        js = np.nonzero(sub_id == sg)[0]
        first[js[0]] = True
        last[js[-1]] = True

    per_core = []
    for c in range(NCORES):
        m = core == c
        rc = r[m]
        clm = cl[m]
        lc = lcol[m]
        tc_ = tt[m]
        kc = k[m]
        pic = pi[m]
        prc = pr[m]
        skey = (tc_ * NCHUNKS + kc) * 2 + pic
        order = np.argsort(skey, kind="stable")
        sorted_key = skey[order]
        firsts = np.searchsorted(sorted_key, sorted_key, side="left")
        ranks = np.arange(len(sorted_key)) - firsts
        pos = slot_off[tc_[order], kc[order], pic[order]] + ranks

        idx_stream = np.zeros(e_pad, np.int16)
        tloc_stream = np.zeros(e_pad, np.float32)
        dcol_stream = np.zeros(e_pad, np.float32)
        idx_stream[pos] = (prc[order] - kc[order] * SRC_CHUNK).astype(np.int16)
        tloc_stream[pos] = (lc[order] & 127).astype(np.float32)
        dcol_stream[pos] = ((1.0 - ALPHA) * dinv[clm[order]]).astype(np.float32)

        idx_w = np.zeros((128, e_pad // 16), np.int16)
        for g in range(ngroups):
            for (kk, s, n) in calls_by_group[g]:
                blk = idx_stream[s:s + n].reshape(n // 16, 16).T
                idx_w[:, s // 16:(s + n) // 16] = np.tile(blk, (8, 1))
        tloc_arr = np.ascontiguousarray(tloc_stream.reshape(nchk, 128).T)
        dcol_arr = np.ascontiguousarray(dcol_stream.reshape(nchk, 128).T)
        dv = np.zeros(NPC_PAD, np.float32)
        dv[:NPC] = dinv[c * NPC:(c + 1) * NPC]
        dinvo = np.ascontiguousarray(dv.reshape(TPC, 128).T)
        per_core.append(
            dict(idx=idx_w, tloc=tloc_arr, dcol=dcol_arr, dinvo=dinvo)
        )

    struct = dict(
        e_pad=e_pad,
        nchk=nchk,
        ngroups=ngroups,
        calls_by_group=calls_by_group,
        chunk_tt=chunk_tt,
        chunk_par=chunk_par,
        first=first,
        last=last,
    )
    return struct, per_core


def _build_program(struct):
    import concourse.bass as bass
    import concourse.mybir as mybir
    import concourse.tile as tile
    import concourse.bacc as bacc

    dt = mybir.dt
    f32 = dt.float32
    bf16 = dt.bfloat16
    Alu = mybir.AluOpType
    Act = mybir.ActivationFunctionType

    e_pad = struct["e_pad"]
    nchk = struct["nchk"]
    ngroups = struct["ngroups"]
    calls_by_group = struct["calls_by_group"]
    chunk_tt = struct["chunk_tt"]
    chunk_par = struct["chunk_par"]
    first = struct["first"]
    last = struct["last"]
    rg = [list(range(NCORES))]

    nc = bacc.Bacc("TRN2", target_bir_lowering=False, debug=False,
                   num_devices=NCORES)

    x_in = nc.dram_tensor("x", [NPC_PAD, N_FEATS], f32, kind="ExternalInput")
    idx_in = nc.dram_tensor("idx", [128, e_pad // 16], dt.int16,
                            kind="ExternalInput")
    tloc_in = nc.dram_tensor("tloc", [128, nchk], f32, kind="ExternalInput")
    dcol_in = nc.dram_tensor("dcol", [128, nchk], f32, kind="ExternalInput")
    dinvo_in = nc.dram_tensor("dinvo", [128, TPC], f32, kind="ExternalInput")
    w0_in = nc.dram_tensor("w0", [N_FEATS, HIDDEN], bf16, kind="ExternalInput")
    b0_in = nc.dram_tensor("b0", [HIDDEN, 1], f32, kind="ExternalInput")
    wp_in = nc.dram_tensor("wp", [HIDDEN, N_LAYERS * HIDDEN], bf16,
                           kind="ExternalInput")
    wout_in = nc.dram_tensor("wout", [HIDDEN, N_CLASSES], bf16,
                             kind="ExternalInput")
    bout_in = nc.dram_tensor("bout", [N_CLASSES, 1], f32, kind="ExternalInput")
    iota_in = nc.dram_tensor("iota", [128, 128], bf16, kind="ExternalInput")
    identb_in = nc.dram_tensor("identb", [128, 128], bf16,
                               kind="ExternalInput")
    identf_in = nc.dram_tensor("identf", [128, 128], f32,
                               kind="ExternalInput")
    out_dram = nc.dram_tensor("out", [NPC_PAD, N_CLASSES], f32,
                              kind="ExternalOutput")

    with tile.TileContext(nc) as tc:
        with (
            tc.tile_pool(name="persist", bufs=1) as P,
            tc.tile_pool(name="work", bufs=3) as S,
            tc.tile_pool(name="msgp", bufs=3) as MSG,
            tc.tile_pool(name="ohp", bufs=8) as OH,
            tc.tile_pool(name="psagg", bufs=4, space="PSUM") as PSA,
            tc.tile_pool(name="psmisc", bufs=2, space="PSUM") as PSM,
            tc.tile_pool(name="dram", bufs=1, space="DRAM") as DR,
        ):
            def pload(name, src, shape, dtype=f32):
                t = P.tile(shape, dtype, tag=name)
                nc.sync.dma_start(t[:], src[:])
                return t

            iota = pload("iota", iota_in, [128, 128], bf16)
            identb = pload("identb", identb_in, [128, 128], bf16)
            identf = pload("identf", identf_in, [128, 128])
            w0 = pload("w0", w0_in, [N_FEATS, HIDDEN], bf16)
            b0 = pload("b0", b0_in, [HIDDEN, 1])
            wp = pload("wp", wp_in, [HIDDEN, N_LAYERS * HIDDEN], bf16)
            wout = pload("wout", wout_in, [HIDDEN, N_CLASSES], bf16)
            bout = pload("bout", bout_in, [N_CLASSES, 1])
            tloc = pload("tloc", tloc_in, [128, nchk])
            dcol = pload("dcol", dcol_in, [128, nchk])
            dinvo = pload("dinvo", dinvo_in, [128, TPC])
            h0s = P.tile([HIDDEN, NPC_PAD], bf16, tag="h0s")

            hs_shard = [
                DR.tile([NPC_PAD, HIDDEN], bf16, tag=f"shard{j}",
                        name=f"hs_shard{j}")
                for j in range(N_LAYERS)
            ]
            hs_full = [
                DR.tile([NPAIRS, 128], bf16, tag=f"full{j}",
                        addr_space="Shared", name=f"hs_full{j}")
                for j in range(N_LAYERS)
            ]

            # ---------------- layer 0: h0 = relu(x @ W0 + b0) ----------------
            for t in range(TPC):
                xt = S.tile([128, N_FEATS], bf16, tag="xt")
                nc.gpsimd.dma_start(xt[:], x_in[t * 128:(t + 1) * 128, :])
                xT_ps = PSM.tile([N_FEATS, 128], f32, tag="tp")
                nc.tensor.transpose(xT_ps[:], xt[:], identb[:])
                xT = S.tile([N_FEATS, 128], bf16, tag="xT")
                nc.vector.tensor_copy(xT[:], xT_ps[:])
                h_ps = PSM.tile([HIDDEN, 128], f32, tag="dense")
                nc.tensor.matmul(h_ps[:], lhsT=w0[:], rhs=xT[:],
                                 start=True, stop=True)
                h0t = S.tile([HIDDEN, 128], bf16, tag="h0t")
                nc.scalar.activation(h0t[:], h_ps[:], Act.Relu, bias=b0[:])
                nc.vector.tensor_scalar_mul(
                    h0s[:, t * 128:(t + 1) * 128], h0t[:], ALPHA)
                tp_ps = PSM.tile([128, HIDDEN], f32, tag="tp")
                nc.tensor.transpose(tp_ps[:], h0t[:],
                                    identb[:HIDDEN, :HIDDEN])
                hs_sb = S.tile([128, HIDDEN], bf16, tag="hs")
                nc.vector.tensor_scalar(hs_sb[:], tp_ps[:],
                                        dinvo[:, t:t + 1], None,
                                        op0=Alu.mult)
                nc.sync.dma_start(
                    hs_shard[0][t * 128:(t + 1) * 128, :], hs_sb[:])
            nc.gpsimd.collective_compute(
                "AllGather", Alu.bypass, replica_groups=rg,
                ins=[hs_shard[0].opt()], outs=[hs_full[0].opt()])

            # ---------------- GCNII layers ----------------
            for li in range(N_LAYERS):
                cur = hs_full[li]
                is_last = li == N_LAYERS - 1
                nxt_shard = hs_shard[li + 1] if not is_last else None
                for g in range(ngroups):
                    tts = list(range(g * GROUP_TT,
                                     min((g + 1) * GROUP_TT, TPC)))
                    subs = [tts[i:i + SUB_TT]
                            for i in range(0, len(tts), SUB_TT)]
                    ps_tiles = [PSA.tile([HIDDEN, SUB_TT * 128], f32,
                                         tag="agg", name=f"agg{li}_{g}_{si}")
                                for si in range(len(subs))]
                    for (kk, s, n) in calls_by_group[g]:
                        idx_t = S.tile([128, n // 16], dt.int16, tag="idxs")
                        nc.sync.dma_start(
                            idx_t[:], idx_in[:, s // 16:(s + n) // 16])
                        msg = MSG.tile([128, n // 128, 128], bf16,
                                       tag="msg")
                        rows_k = min(SRC_CHUNK, NPAIRS - kk * SRC_CHUNK)
                        nc.gpsimd.dma_gather(
                            msg[:],
                            cur[kk * SRC_CHUNK:kk * SRC_CHUNK + rows_k, :],
                            idx_t[:], num_idxs=n, num_idxs_reg=n,
                            elem_size=128, single_packet=False)
                        for jj in range(n // 128):
                            j = s // 128 + jj
                            t = int(chunk_tt[j])
                            pp = int(chunk_par[j])
                            oh = OH.tile([128, 128], bf16, tag="oh")
                            nc.vector.tensor_scalar(
                                oh[:], iota[:], tloc[:, j:j + 1],
                                dcol[:, j:j + 1],
                                op0=Alu.is_equal, op1=Alu.mult)
                            si = (t - g * GROUP_TT) // SUB_TT
                            ci = (t % SUB_TT) * 128
                            nc.tensor.matmul(
                                ps_tiles[si][:, ci:ci + 128],
                                lhsT=msg[:, jj, pp * 64:pp * 64 + 64],
                                rhs=oh[:],
                                start=bool(first[j]), stop=bool(last[j]),
                                skip_group_check=True)
                    for si, stts in enumerate(subs):
                        w = len(stts) * 128
                        n0 = stts[0] * 128
                        sup = S.tile([HIDDEN, SUB_TT * 128], bf16, tag="sup")
                        nc.vector.tensor_tensor(
                            out=sup[:, :w], in0=ps_tiles[si][:, :w],
                            in1=h0s[:, n0:n0 + w], op=Alu.add)
                        d_ps = PSM.tile([HIDDEN, SUB_TT * 128], f32,
                                        tag="dense")
                        nc.tensor.matmul(
                            d_ps[:, :w],
                            lhsT=wp[:, li * HIDDEN:(li + 1) * HIDDEN],
                            rhs=sup[:, :w], start=True, stop=True)
                        ht = S.tile([HIDDEN, SUB_TT * 128], bf16, tag="ht")
                        nc.scalar.activation(ht[:, :w], d_ps[:, :w], Act.Relu)
                        if not is_last:
                            for tti, t in enumerate(stts):
                                tp_ps = PSM.tile([128, HIDDEN], f32,
                                                 tag="tp")
                                nc.tensor.transpose(
                                    tp_ps[:],
                                    ht[:, tti * 128:(tti + 1) * 128],
                                    identb[:HIDDEN, :HIDDEN])
                                hs_sb = S.tile([128, HIDDEN], bf16, tag="hs")
                                nc.vector.tensor_scalar(
                                    hs_sb[:], tp_ps[:], dinvo[:, t:t + 1],
                                    None, op0=Alu.mult)
                                nc.sync.dma_start(
                                    nxt_shard[t * 128:(t + 1) * 128, :],
                                    hs_sb[:])
                        else:
                            o_ps = PSM.tile([N_CLASSES, SUB_TT * 128], f32,
                                            tag="dense")
                            nc.tensor.matmul(o_ps[:, :w], lhsT=wout[:],
                                             rhs=ht[:, :w],
                                             start=True, stop=True)
                            o_sb = S.tile([N_CLASSES, SUB_TT * 128], f32,
                                          tag="osb")
                            nc.vector.tensor_scalar(
                                o_sb[:, :w], o_ps[:, :w], bout[:], None,
                                op0=Alu.add)
                            for tti, t in enumerate(stts):
                                tp_ps = PSM.tile([128, HIDDEN], f32,
                                                 tag="tp")
                                nc.tensor.transpose(
                                    tp_ps[:, :N_CLASSES],
                                    o_sb[:, tti * 128:(tti + 1) * 128],
                                    identf[:N_CLASSES, :N_CLASSES])
                                ot = S.tile([128, N_CLASSES], f32, tag="ot")
                                nc.vector.tensor_copy(
                                    ot[:], tp_ps[:, :N_CLASSES])
                                nc.sync.dma_start(
                                    out_dram[t * 128:(t + 1) * 128, :],
                                    ot[:])
                if not is_last:
                    nc.gpsimd.collective_compute(
                        "AllGather", Alu.bypass, replica_groups=rg,
                        ins=[nxt_shard.opt()],
                        outs=[hs_full[li + 1].opt()])

    nc.compile()
    return nc


def kernel(x, edge_index, W0, b0, Wl, W_out, b_out):
    from concourse.bass_utils import run_bass_kernel_spmd
    from concourse import mybir

    bf16np = mybir.dt.np(mybir.dt.bfloat16)

    x = np.asarray(x, dtype=np.float32)
    edge_index = np.asarray(edge_index)
    W0 = np.asarray(W0, dtype=np.float32)
    b0 = np.asarray(b0, dtype=np.float32)
    Wl = np.asarray(Wl, dtype=np.float32)
    W_out = np.asarray(W_out, dtype=np.float32)
    b_out = np.asarray(b_out, dtype=np.float32)

    struct, per_core = _preprocess(edge_index)
    nc = _build_program(struct)

    betas = np.array(
        [math.log(LAMDA / (i + 1) + 1.0) for i in range(N_LAYERS)],
        dtype=np.float32)
    wp_host = np.zeros((HIDDEN, N_LAYERS * HIDDEN), np.float32)
    eye = np.eye(HIDDEN, dtype=np.float32)
    for i in range(N_LAYERS):
        wp_host[:, i * HIDDEN:(i + 1) * HIDDEN] = (
            betas[i] * Wl[i] + (1.0 - betas[i]) * eye)
    iota_host = np.tile(np.arange(128, dtype=np.float32), (128, 1))
    ident_host = np.eye(128, dtype=np.float32)

    in_maps = []
    for c in range(NCORES):
        xc = np.zeros((NPC_PAD, N_FEATS), np.float32)
        xc[:NPC] = x[c * NPC:(c + 1) * NPC]
        pc = per_core[c]
        in_maps.append({
            "x": xc,
            "idx": pc["idx"],
            "tloc": pc["tloc"],
            "dcol": pc["dcol"],
            "dinvo": pc["dinvo"],
            "w0": W0.astype(bf16np),
            "b0": b0.reshape(HIDDEN, 1),
            "wp": wp_host.astype(bf16np),
            "wout": W_out.astype(bf16np),
            "bout": b_out.reshape(N_CLASSES, 1),
            "iota": iota_host.astype(bf16np),
            "identb": ident_host.astype(bf16np),
            "identf": ident_host,
        })

    res = run_bass_kernel_spmd(
        nc, in_maps, core_ids=list(range(NCORES)), trace=TRACE)
    _LAST_RESULT["res"] = res
    out = np.concatenate(
        [res.results[c]["out"][:NPC] for c in range(NCORES)], axis=0)
    return out


# revision 14
# speedup vs baseline: 1.7431x; 1.7431x over previous
"""GCNII (nn_GCNII_17626545783193) Bass/Trainium2 kernel, 8 NeuronCores.

Strategy (target-node sharding, feature-major compute, bf16 aggregation):
  - Nodes sharded 12500/core (padded to 12544 = 98*128). Edges partitioned
    by target core. Self-loops are NOT gathered: they are folded in as one
    diagonal one-hot matmul per target tile reading the core's own
    node-major hs tile from SBUF.
  - Hidden states hs = dinv*h stored in bf16 with DUPLICATED halves:
    hs_full is [100352, 128] bf16 where row n = [h_n | h_n] (the dma_gather
    stride/elem floor is 256B). The matmul stationary slices cols 0:64.
  - gcn_norm factored: norm[e] = dinv[src]*dinv[dst]; dinv[src] folded into
    hs, 0.9*dinv[dst] folded into the one-hot scatter matrix (dcol).
  - Per layer, per core: dma_gather of in-edge source rows from hs_full
    across 4 SWDGE queues (overlapped drains; ~5.4ns/idx); DVE builds bf16
    one-hot [128edge, 128target] tiles (iota==tloc)*dcol reading iota from
    PSUM (1-port mode - avoids the DVE<->GpSimd shared-SBUF-port lock that
    would serialize one-hot builds against SWDGE descriptor generation);
    PE computes agg^T = msgs^T @ onehot (bf16, 1 cyc/row) accumulated in
    PSUM [64, 512]; support = agg + 0.1*h0^T (DVE); GCNII dense update via
    ONE matmul with host-fused W' = beta*Wl + (1-beta)*I; ACT relu; PE
    transpose back to node-major; scale by dinv into a persistent SBUF
    hs_own tile (feeds next layer's diagonal term); DMA both 64-col halves
    to the shard; AllGather.
"""
import sys
sys.path.insert(0, "/opt/trn_rl_repo")

import math
import numpy as np

# ---- problem constants (hardcoded per spec) ----
N_NODES = 100000
N_FEATS = 128
HIDDEN = 64
N_CLASSES = 40
N_LAYERS = 8
ALPHA = 0.1
LAMDA = 0.5
NCORES = 8

NPC = N_NODES // NCORES            # 12500 real nodes per core
TPC = (NPC + 127) // 128           # 98 target tiles per core
NPC_PAD = TPC * 128                # 12544
N_PAD = NPC_PAD * NCORES           # 100352
GROUP_TT = 8                       # target tiles per gather group
SUB_TT = 4                         # target tiles per psum tile (512 cols)
SRC_CHUNK = 32768                  # int16 index range per gather source chunk
NCHUNKS = (N_PAD + SRC_CHUNK - 1) // SRC_CHUNK  # 4
CALL_MAX = 8192                    # max indices per dma_gather call
NQUEUES = 4                        # SWDGE queues (overlapped gather drains)

TRACE = False          # test.py sets this for profiling
_LAST_RESULT = {}      # test.py reads exec_time from here


def _preprocess(edge_index):
    """Build the uniform (cross-core identical) edge stream structure and the
    per-core data arrays.  Self-loops are excluded from the gather stream
    (handled as diagonal matmuls on-chip)."""
    row = np.asarray(edge_index[0], dtype=np.int64)
    col = np.asarray(edge_index[1], dtype=np.int64)
    deg = np.bincount(col, minlength=N_NODES).astype(np.float32) + 1.0
    dinv = (1.0 / np.sqrt(deg)).astype(np.float32)

    r = row
    cl = col
    core = cl // NPC
    lcol = cl - core * NPC                       # 0..12499 local target
    tt = lcol >> 7                               # target tile 0..97
    gp = (r // NPC) * NPC_PAD + (r % NPC)        # padded global source id
    k = gp // SRC_CHUNK                          # source chunk 0..3

    ngroups = (TPC + GROUP_TT - 1) // GROUP_TT

    slot = (core * TPC + tt) * NCHUNKS + k
    counts = np.bincount(slot, minlength=NCORES * TPC * NCHUNKS).reshape(
        NCORES, TPC, NCHUNKS
    )
    maxc = counts.max(axis=0)                    # [TPC, NCHUNKS]
    slot_chunks = -(-maxc // 128)                # ceil; 0 allowed
    slot_len = slot_chunks * 128

    slot_off = np.zeros((TPC, NCHUNKS), np.int64)
    off = 0
    chunk_tt = []
    calls_by_group = [[] for _ in range(ngroups)]
    for g in range(ngroups):
        tts = range(g * GROUP_TT, min((g + 1) * GROUP_TT, TPC))
        for kk in range(NCHUNKS):
            run_start = off
            for t in tts:
                slot_off[t, kk] = off
                off += slot_len[t, kk]
                chunk_tt += [t] * slot_chunks[t, kk]
            s = run_start
            while s < off:
                n = min(CALL_MAX, off - s)
                calls_by_group[g].append((kk, int(s), int(n)))
                s += n
    e_pad = off
    nchk = e_pad // 128
    chunk_tt = np.asarray(chunk_tt, np.int64)
    # start/stop are per PSUM tile (= subgroup of SUB_TT ttiles): exactly one
    # start=True (chronologically first matmul into the bank) and one
    # stop=True (last). The diagonal (self-loop) matmul is appended after the
    # group's gather chunks and is always chronologically last for its sub,
    # so it carries stop=True and the last gather chunk's stop is cleared.
    sub_id = chunk_tt // SUB_TT
    first = np.zeros(nchk, bool)
    last = np.zeros(nchk, bool)   # all False: diag matmuls carry stop
    for sg in np.unique(sub_id):
        js = np.nonzero(sub_id == sg)[0]
        first[js[0]] = True

    per_core = []
    for c in range(NCORES):
        m = core == c
        rc = r[m]
        clm = cl[m]
        lc = lcol[m]
        tc_ = tt[m]
        kc = k[m]
        gpc = gp[m]
        # sort by slot, then ascending source id within the slot (possible
        # HBM locality for the gather engines)
        skey = tc_ * NCHUNKS + kc
        order = np.lexsort((gpc, skey))
        sorted_key = skey[order]
        firsts = np.searchsorted(sorted_key, sorted_key, side="left")
        ranks = np.arange(len(sorted_key)) - firsts
        pos = slot_off[tc_[order], kc[order]] + ranks

        idx_stream = np.zeros(e_pad, np.int16)
        tloc_stream = np.zeros(e_pad, np.float32)
        dcol_stream = np.zeros(e_pad, np.float32)
        idx_stream[pos] = (gpc[order] - kc[order] * SRC_CHUNK).astype(np.int16)
        tloc_stream[pos] = (lc[order] & 127).astype(np.float32)
        dcol_stream[pos] = ((1.0 - ALPHA) * dinv[clm[order]]).astype(np.float32)

        idx_w = np.zeros((128, e_pad // 16), np.int16)
        for g in range(ngroups):
            for (kk, s, n) in calls_by_group[g]:
                blk = idx_stream[s:s + n].reshape(n // 16, 16).T
                idx_w[:, s // 16:(s + n) // 16] = np.tile(blk, (8, 1))
        # diag (self-loop) columns appended at nchk + t: tloc = partition id,
        # dcol = 0.9 * dinv^2 of the tile's local nodes
        dvl = np.zeros(NPC_PAD, np.float32)
        dvl[:NPC] = dinv[c * NPC:(c + 1) * NPC]
        diag_tloc = np.tile(np.arange(128, dtype=np.float32), (TPC, 1))
        # hs_own rows already carry one dinv factor; dcol supplies the other
        diag_dcol = (1.0 - ALPHA) * dvl.reshape(TPC, 128)
        tloc_arr = np.ascontiguousarray(
            np.concatenate([tloc_stream.reshape(nchk, 128), diag_tloc]).T)
        dcol_arr = np.ascontiguousarray(
            np.concatenate([dcol_stream.reshape(nchk, 128), diag_dcol]).T)
        dinvo = np.ascontiguousarray(dvl.reshape(TPC, 128).T)
        per_core.append(
            dict(idx=idx_w, tloc=tloc_arr, dcol=dcol_arr, dinvo=dinvo)
        )

    struct = dict(
        e_pad=e_pad,
        nchk=nchk,
        ngroups=ngroups,
        calls_by_group=calls_by_group,
        chunk_tt=chunk_tt,
        first=first,
        last=last,
    )
    return struct, per_core


def _build_program(struct):
    import concourse.bass as bass
    import concourse.mybir as mybir
    import concourse.tile as tile
    import concourse.bacc as bacc

    dt = mybir.dt
    f32 = dt.float32
    bf16 = dt.bfloat16
    Alu = mybir.AluOpType
    Act = mybir.ActivationFunctionType

    e_pad = struct["e_pad"]
    nchk = struct["nchk"]
    ngroups = struct["ngroups"]
    calls_by_group = struct["calls_by_group"]
    chunk_tt = struct["chunk_tt"]
    first = struct["first"]
    rg = [list(range(NCORES))]

    nc = bacc.Bacc("TRN2", target_bir_lowering=False, debug=False,
                   num_devices=NCORES, num_swdge_queues=NQUEUES)

    xT_in = nc.dram_tensor("xT", [N_FEATS, NPC_PAD], f32, kind="ExternalInput")
    idx_in = nc.dram_tensor("idx", [128, e_pad // 16], dt.int16,
                            kind="ExternalInput")
    tloc_in = nc.dram_tensor("tloc", [128, nchk + TPC], f32,
                             kind="ExternalInput")
    dcol_in = nc.dram_tensor("dcol", [128, nchk + TPC], f32,
                             kind="ExternalInput")
    dinvo_in = nc.dram_tensor("dinvo", [128, TPC], f32, kind="ExternalInput")
    w0_in = nc.dram_tensor("w0", [N_FEATS, HIDDEN], f32, kind="ExternalInput")
    b0_in = nc.dram_tensor("b0", [HIDDEN, 1], f32, kind="ExternalInput")
    wp_in = nc.dram_tensor("wp", [HIDDEN, N_LAYERS * HIDDEN], bf16,
                           kind="ExternalInput")
    woute_in = nc.dram_tensor("woute", [HIDDEN + 1, N_CLASSES], bf16,
                              kind="ExternalInput")
    iota_in = nc.dram_tensor("iota", [128, 128], f32, kind="ExternalInput")
    identb_in = nc.dram_tensor("identb", [128, 128], bf16,
                               kind="ExternalInput")
    out_dram = nc.dram_tensor("out", [NPC_PAD, N_CLASSES], f32,
                              kind="ExternalOutput")

    with tile.TileContext(nc) as tc:
        with (
            tc.tile_pool(name="persist", bufs=1) as P,
            tc.tile_pool(name="work", bufs=3) as S,
            tc.tile_pool(name="msgp", bufs=4) as MSG,
            tc.tile_pool(name="ohp", bufs=8) as OH,
            tc.tile_pool(name="psagg", bufs=3, space="PSUM") as PSA,
            tc.tile_pool(name="psmisc", bufs=2, space="PSUM") as PSM,
            tc.tile_pool(name="pstp", bufs=2, space="PSUM") as PST,
            tc.tile_pool(name="pspers", bufs=1, space="PSUM") as PP,
            tc.tile_pool(name="dram", bufs=1, space="DRAM") as DR,
        ):
            def pload(name, src, shape, dtype=f32):
                t = P.tile(shape, dtype, tag=name)
                nc.sync.dma_start(t[:], src[:])
                return t

            iota_sb = pload("iota", iota_in, [128, 128])
            identb = pload("identb", identb_in, [128, 128], bf16)
            w0 = pload("w0", w0_in, [N_FEATS, HIDDEN])
            b0 = pload("b0", b0_in, [HIDDEN, 1])
            wp = pload("wp", wp_in, [HIDDEN, N_LAYERS * HIDDEN], bf16)
            woute = pload("woute", woute_in, [HIDDEN + 1, N_CLASSES], bf16)
            tloc = pload("tloc", tloc_in, [128, nchk + TPC])
            dcol = pload("dcol", dcol_in, [128, nchk + TPC])
            dinvo = pload("dinvo", dinvo_in, [128, TPC])
            h0s = P.tile([HIDDEN, NPC_PAD], bf16, tag="h0s")
            hs_own = P.tile([128, TPC * HIDDEN], bf16, tag="hs_own")
            # iota kept in PSUM so one-hot builds read via the PSUM port
            # (1x mode, single SBUF write port): no DVE<->GpSimd SBUF-port
            # lock, so they overlap with SWDGE descriptor generation.
            iota_ps_full = PP.tile([128, 512], f32, tag="iops")
            iota_ps = iota_ps_full[:, 0:128]
            nc.vector.tensor_copy(iota_ps, iota_sb[:])

            hs_shard = [
                DR.tile([NPC_PAD, 128], bf16, tag=f"shard{j}",
                        name=f"hs_shard{j}")
                for j in range(N_LAYERS)
            ]
            hs_full = [
                DR.tile([N_PAD, 128], bf16, tag=f"full{j}",
                        addr_space="Shared", name=f"hs_full{j}")
                for j in range(N_LAYERS)
            ]

            def emit_hs_out(src_ps, t, shard):
                """dinv-scale a node-major PSUM tile into hs_own and DMA
                both duplicated halves to the shard."""
                dst = hs_own[:, t * HIDDEN:(t + 1) * HIDDEN]
                nc.vector.tensor_scalar(dst, src_ps[:], dinvo[:, t:t + 1],
                                        None, op0=Alu.mult)
                nc.sync.dma_start(
                    shard[t * 128:(t + 1) * 128, 0:HIDDEN], dst)
                nc.sync.dma_start(
                    shard[t * 128:(t + 1) * 128, HIDDEN:128], dst)

            # ---------------- layer 0: h0 = relu(x @ W0 + b0) ----------------
            for t in range(TPC):
                xT = S.tile([N_FEATS, 128], f32, tag="xT")
                nc.sync.dma_start(xT[:], xT_in[:, t * 128:(t + 1) * 128])
                h_ps = PSM.tile([HIDDEN, 128], f32, tag="dense")
                nc.tensor.matmul(h_ps[:], lhsT=w0[:], rhs=xT[:],
                                 start=True, stop=True)
                h0t = S.tile([HIDDEN, 128], bf16, tag="h0t")
                nc.scalar.activation(h0t[:], h_ps[:], Act.Relu, bias=b0[:])
                nc.scalar.activation(
                    h0s[:, t * 128:(t + 1) * 128], h0t[:], Act.Copy,
                    scale=ALPHA)
                tp_ps = PST.tile([128, HIDDEN], bf16, tag="tpb")
                nc.tensor.transpose(tp_ps[:], h0t[:],
                                    identb[:HIDDEN, :HIDDEN])
                emit_hs_out(tp_ps, t, hs_shard[0])
            nc.gpsimd.collective_compute(
                "AllGather", Alu.bypass, replica_groups=rg,
                ins=[hs_shard[0].opt()], outs=[hs_full[0].opt()])

            # ---------------- GCNII layers ----------------
            call_no = 0
            for li in range(N_LAYERS):
                cur = hs_full[li]
                is_last = li == N_LAYERS - 1
                nxt_shard = hs_shard[li + 1] if not is_last else None
                for g in range(ngroups):
                    tts = list(range(g * GROUP_TT,
                                     min((g + 1) * GROUP_TT, TPC)))
                    subs = [tts[i:i + SUB_TT]
                            for i in range(0, len(tts), SUB_TT)]
                    ps_tiles = [PSA.tile([HIDDEN, SUB_TT * 128], f32,
                                         tag="agg", name=f"agg{li}_{g}_{si}")
                                for si in range(len(subs))]

                    def onehot(j):
                        oh = OH.tile([128, 128], bf16, tag="oh")
                        nc.vector.tensor_scalar(
                            oh[:], iota_ps[:], tloc[:, j:j + 1],
                            dcol[:, j:j + 1],
                            op0=Alu.is_equal, op1=Alu.mult)
                        return oh

                    for (kk, s, n) in calls_by_group[g]:
                        idx_t = S.tile([128, n // 16], dt.int16, tag="idxs")
                        nc.sync.dma_start(
                            idx_t[:], idx_in[:, s // 16:(s + n) // 16])
                        msg = MSG.tile([128, n // 128, 128], bf16,
                                       tag="msg")
                        rows_k = min(SRC_CHUNK, N_PAD - kk * SRC_CHUNK)
                        nc.gpsimd.dma_gather(
                            msg[:],
                            cur[kk * SRC_CHUNK:kk * SRC_CHUNK + rows_k, :],
                            idx_t[:], num_idxs=n, num_idxs_reg=n,
                            elem_size=128, single_packet=False,
                            queue_num=call_no % NQUEUES)
                        call_no += 1
                        for jj in range(n // 128):
                            j = s // 128 + jj
                            t = int(chunk_tt[j])
                            oh = onehot(j)
                            si = (t - g * GROUP_TT) // SUB_TT
                            ci = (t % SUB_TT) * 128
                            nc.tensor.matmul(
                                ps_tiles[si][:, ci:ci + 128],
                                lhsT=msg[:, jj, 0:HIDDEN],
                                rhs=oh[:],
                                start=bool(first[j]), stop=False,
                                skip_group_check=True)
                    # diagonal (self-loop) term: one matmul per target tile
                    # reading the core's own node-major hs tile from SBUF
                    for t in tts:
                        oh = onehot(nchk + t)
                        si = (t - g * GROUP_TT) // SUB_TT
                        ci = (t % SUB_TT) * 128
                        nc.tensor.matmul(
                            ps_tiles[si][:, ci:ci + 128],
                            lhsT=hs_own[:, t * HIDDEN:(t + 1) * HIDDEN],
                            rhs=oh[:],
                            start=False, stop=(t % SUB_TT == SUB_TT - 1
                                               or t == tts[-1]),
                            skip_group_check=True)
                    for si, stts in enumerate(subs):
                        w = len(stts) * 128
                        n0 = stts[0] * 128
                        sup = S.tile([HIDDEN, SUB_TT * 128], bf16, tag="sup")
                        nc.vector.tensor_tensor(
                            out=sup[:, :w], in0=ps_tiles[si][:, :w],
                            in1=h0s[:, n0:n0 + w], op=Alu.add)
                        d_ps = PSM.tile([HIDDEN, SUB_TT * 128], f32,
                                        tag="dense")
                        nc.tensor.matmul(
                            d_ps[:, :w],
                            lhsT=wp[:, li * HIDDEN:(li + 1) * HIDDEN],
                            rhs=sup[:, :w], start=True, stop=True)
                        ht = S.tile([HIDDEN + 1, SUB_TT * 128], bf16,
                                    tag="ht")
                        nc.scalar.activation(ht[:HIDDEN, :w], d_ps[:, :w],
                                             Act.Relu)
                        if not is_last:
                            for tti, t in enumerate(stts):
                                tp_ps = PST.tile([128, HIDDEN], bf16,
                                                 tag="tpb")
                                nc.tensor.transpose(
                                    tp_ps[:],
                                    ht[:HIDDEN, tti * 128:(tti + 1) * 128],
                                    identb[:HIDDEN, :HIDDEN])
                                emit_hs_out(tp_ps, t, nxt_shard)
                        else:
                            # node-major output: out = ht^T @ [W_out; b_out]
                            # (bias folded as a ones row in ht)
                            nc.vector.memset(ht[HIDDEN:HIDDEN + 1, :w], 1.0)
                            for tti, t in enumerate(stts):
                                o_ps = PSM.tile([128, N_CLASSES], f32,
                                                tag="dense")
                                nc.tensor.matmul(
                                    o_ps[:],
                                    lhsT=ht[:, tti * 128:(tti + 1) * 128],
                                    rhs=woute[:],
                                    start=True, stop=True)
                                ot = S.tile([128, N_CLASSES], f32, tag="ot")
                                nc.vector.tensor_copy(ot[:], o_ps[:])
                                nc.sync.dma_start(
                                    out_dram[t * 128:(t + 1) * 128, :],
                                    ot[:])
                if not is_last:
                    nc.gpsimd.collective_compute(
                        "AllGather", Alu.bypass, replica_groups=rg,
                        ins=[nxt_shard.opt()],
                        outs=[hs_full[li + 1].opt()])

    nc.compile()
    return nc


def kernel(x, edge_index, W0, b0, Wl, W_out, b_out):
    from concourse.bass_utils import run_bass_kernel_spmd
    from concourse import mybir

    bf16np = mybir.dt.np(mybir.dt.bfloat16)

    x = np.asarray(x, dtype=np.float32)
    edge_index = np.asarray(edge_index)
    W0 = np.asarray(W0, dtype=np.float32)
    b0 = np.asarray(b0, dtype=np.float32)
    Wl = np.asarray(Wl, dtype=np.float32)
    W_out = np.asarray(W_out, dtype=np.float32)
    b_out = np.asarray(b_out, dtype=np.float32)

    struct, per_core = _preprocess(edge_index)
    nc = _build_program(struct)

    betas = np.array(
        [math.log(LAMDA / (i + 1) + 1.0) for i in range(N_LAYERS)],
        dtype=np.float32)
    wp_host = np.zeros((HIDDEN, N_LAYERS * HIDDEN), np.float32)
    eye = np.eye(HIDDEN, dtype=np.float32)
    for i in range(N_LAYERS):
        wp_host[:, i * HIDDEN:(i + 1) * HIDDEN] = (
            betas[i] * Wl[i] + (1.0 - betas[i]) * eye)
    woute_host = np.concatenate(
        [W_out, b_out.reshape(1, N_CLASSES)], axis=0)
    iota_host = np.tile(np.arange(128, dtype=np.float32), (128, 1))
    ident_host = np.eye(128, dtype=np.float32)

    in_maps = []
    for c in range(NCORES):
        xTc = np.zeros((N_FEATS, NPC_PAD), np.float32)
        xTc[:, :NPC] = x[c * NPC:(c + 1) * NPC].T
        pc = per_core[c]
        in_maps.append({
            "xT": xTc,
            "idx": pc["idx"],
            "tloc": pc["tloc"],
            "dcol": pc["dcol"],
            "dinvo": pc["dinvo"],
            "w0": W0,
            "b0": b0.reshape(HIDDEN, 1),
            "wp": wp_host.astype(bf16np),
            "woute": woute_host.astype(bf16np),
            "iota": iota_host,
            "identb": ident_host.astype(bf16np),
        })

    res = run_bass_kernel_spmd(
        nc, in_maps, core_ids=list(range(NCORES)), trace=TRACE)
    _LAST_RESULT["res"] = res
    out = np.concatenate(
        [res.results[c]["out"][:NPC] for c in range(NCORES)], axis=0)
    return out


# revision 23
# speedup vs baseline: 1.7748x; 1.0182x over previous
"""GCNII (nn_GCNII_17626545783193) Bass/Trainium2 kernel, 8 NeuronCores.

Strategy (target-node sharding, feature-major compute, bf16 aggregation):
  - Nodes sharded 12500/core (padded to 12544 = 98*128). Edges partitioned
    by target core. Self-loops are NOT gathered: they are folded in as one
    diagonal one-hot matmul per target tile reading the core's own
    node-major hs tile from SBUF.
  - Hidden states hs = dinv*h stored in bf16 with DUPLICATED halves:
    hs_full is [100352, 128] bf16 where row n = [h_n | h_n] (the dma_gather
    stride/elem floor is 256B). The matmul stationary slices cols 0:64.
  - gcn_norm factored: norm[e] = dinv[src]*dinv[dst]; dinv[src] folded into
    hs, 0.9*dinv[dst] folded into the one-hot scatter matrix (dcol).
  - Per layer, per core: dma_gather of in-edge source rows from hs_full
    across 4 SWDGE queues (overlapped drains; ~5.4ns/idx); DVE builds bf16
    one-hot [128edge, 128target] tiles (iota==tloc)*dcol reading iota from
    PSUM (1-port mode - avoids the DVE<->GpSimd shared-SBUF-port lock that
    would serialize one-hot builds against SWDGE descriptor generation);
    PE computes agg^T = msgs^T @ onehot (bf16, 1 cyc/row) accumulated in
    PSUM [64, 512]; support = agg + 0.1*h0^T (DVE); GCNII dense update via
    ONE matmul with host-fused W' = beta*Wl + (1-beta)*I; ACT relu; PE
    transpose back to node-major; scale by dinv into a persistent SBUF
    hs_own tile (feeds next layer's diagonal term); DMA both 64-col halves
    to the shard; AllGather.
"""
import sys
sys.path.insert(0, "/opt/trn_rl_repo")

import math
import numpy as np

# ---- problem constants (hardcoded per spec) ----
N_NODES = 100000
N_FEATS = 128
HIDDEN = 64
N_CLASSES = 40
N_LAYERS = 8
ALPHA = 0.1
LAMDA = 0.5
NCORES = 8

NPC = N_NODES // NCORES            # 12500 real nodes per core
TPC = (NPC + 127) // 128           # 98 target tiles per core
NPC_PAD = TPC * 128                # 12544
N_PAD = NPC_PAD * NCORES           # 100352
GROUP_TT = 8                       # target tiles per gather group
SUB_TT = 4                         # target tiles per psum tile (512 cols)
SRC_CHUNK = 32768                  # int16 index range per gather source chunk
NCHUNKS = (N_PAD + SRC_CHUNK - 1) // SRC_CHUNK  # 4
CALL_MAX = 8192                    # max indices per dma_gather call
NQUEUES = 4                        # SWDGE queues (overlapped gather drains)

TRACE = False          # test.py sets this for profiling
_LAST_RESULT = {}      # test.py reads exec_time from here


def _preprocess(edge_index):
    """Build the uniform (cross-core identical) edge stream structure and the
    per-core data arrays.  Self-loops are excluded from the gather stream
    (handled as diagonal matmuls on-chip)."""
    row = np.asarray(edge_index[0], dtype=np.int64)
    col = np.asarray(edge_index[1], dtype=np.int64)
    deg = np.bincount(col, minlength=N_NODES).astype(np.float32) + 1.0
    dinv = (1.0 / np.sqrt(deg)).astype(np.float32)

    r = row
    cl = col
    core = cl // NPC
    lcol = cl - core * NPC                       # 0..12499 local target
    tt = lcol >> 7                               # target tile 0..97
    gp = (r // NPC) * NPC_PAD + (r % NPC)        # padded global source id
    k = gp // SRC_CHUNK                          # source chunk 0..3

    ngroups = (TPC + GROUP_TT - 1) // GROUP_TT

    slot = (core * TPC + tt) * NCHUNKS + k
    counts = np.bincount(slot, minlength=NCORES * TPC * NCHUNKS).reshape(
        NCORES, TPC, NCHUNKS
    )
    maxc = counts.max(axis=0)                    # [TPC, NCHUNKS]
    # 128-granularity slots (one full matmul per chunk; sliced-partition
    # half-chunk matmuls crash the hardware, so no 64-grain packing)
    slot_len = (-(-maxc // 128) * 128).astype(np.int64)

    slot_off = np.zeros((TPC, NCHUNKS), np.int64)
    off = 0
    calls_by_group = [[] for _ in range(ngroups)]
    for g in range(ngroups):
        tts = range(g * GROUP_TT, min((g + 1) * GROUP_TT, TPC))
        for kk in range(NCHUNKS):
            run_start = off
            for t in tts:
                slot_off[t, kk] = off
                off += slot_len[t, kk]
            if (off - run_start) % 128:
                off += 64            # keep calls 128-aligned
            s = run_start
            while s < off:
                n = min(CALL_MAX, off - s)
                calls_by_group[g].append((kk, int(s), int(n)))
                s += n
    e_pad = off
    nchk = e_pad // 128
    # chunk entries: per 128-row chunk, the (tile, row_lo, row_hi) spans of
    # the slots overlapping it (1 normally, 2 when straddling).
    entries = [[] for _ in range(nchk)]
    for t in range(TPC):
        for kk in range(NCHUNKS):
            s0 = int(slot_off[t, kk])
            s1 = s0 + int(slot_len[t, kk])
            j = s0 // 128
            while j * 128 < s1:
                lo = max(s0, j * 128)
                hi = min(s1, (j + 1) * 128)
                entries[j].append((t, lo - j * 128, hi - j * 128))
                j += 1
    # start flags: per PSUM tile (= subgroup of SUB_TT ttiles): exactly one
    # start=True (chronologically first matmul into the bank). The diagonal
    # (self-loop) matmul is appended after the group's gather chunks and is
    # always chronologically last for its sub, so it carries stop=True.
    seen_sub = set()
    entry_start = []
    for j in range(nchk):
        es = []
        for (t, lo, hi) in entries[j]:
            sg = t // SUB_TT
            es.append(sg not in seen_sub)
            seen_sub.add(sg)
        entry_start.append(es)

    per_core = []
    for c in range(NCORES):
        m = core == c
        clm = cl[m]
        lc = lcol[m]
        tc_ = tt[m]
        kc = k[m]
        gpc = gp[m]
        # sort by slot, then ascending source id within the slot (possible
        # HBM locality for the gather engines)
        skey = tc_ * NCHUNKS + kc
        order = np.lexsort((gpc, skey))
        sorted_key = skey[order]
        firsts = np.searchsorted(sorted_key, sorted_key, side="left")
        ranks = np.arange(len(sorted_key)) - firsts
        pos = slot_off[tc_[order], kc[order]] + ranks

        idx_stream = np.zeros(e_pad, np.int16)
        tloc_stream = np.zeros(e_pad, np.float32)
        dcol_stream = np.zeros(e_pad, np.float32)
        idx_stream[pos] = (gpc[order] - kc[order] * SRC_CHUNK).astype(np.int16)
        tloc_stream[pos] = (lc[order] & 127).astype(np.float32)
        dcol_stream[pos] = ((1.0 - ALPHA) * dinv[clm[order]]).astype(np.float32)

        idx_w = np.zeros((128, e_pad // 16), np.int16)
        for g in range(ngroups):
            for (kk, s, n) in calls_by_group[g]:
                blk = idx_stream[s:s + n].reshape(n // 16, 16).T
                idx_w[:, s // 16:(s + n) // 16] = np.tile(blk, (8, 1))
        # diag (self-loop) columns appended at nchk + t: tloc = partition id,
        # dcol = 0.9 * dinv^2 of the tile's local nodes
        dvl = np.zeros(NPC_PAD, np.float32)
        dvl[:NPC] = dinv[c * NPC:(c + 1) * NPC]
        diag_tloc = np.tile(np.arange(128, dtype=np.float32), (TPC, 1))
        # hs_own rows already carry one dinv factor; dcol supplies the other
        diag_dcol = (1.0 - ALPHA) * dvl.reshape(TPC, 128)
        tloc_arr = np.ascontiguousarray(
            np.concatenate([tloc_stream.reshape(nchk, 128), diag_tloc]).T)
        dcol_arr = np.ascontiguousarray(
            np.concatenate([dcol_stream.reshape(nchk, 128), diag_dcol]).T)
        dinvo = np.ascontiguousarray(dvl.reshape(TPC, 128).T)
        per_core.append(
            dict(idx=idx_w, tloc=tloc_arr, dcol=dcol_arr, dinvo=dinvo)
        )

    struct = dict(
        e_pad=e_pad,
        nchk=nchk,
        ngroups=ngroups,
        calls_by_group=calls_by_group,
        entries=entries,
        entry_start=entry_start,
    )
    return struct, per_core


def _build_program(struct):
    import concourse.bass as bass
    import concourse.mybir as mybir
    import concourse.tile as tile
    import concourse.bacc as bacc

    dt = mybir.dt
    f32 = dt.float32
    bf16 = dt.bfloat16
    Alu = mybir.AluOpType
    Act = mybir.ActivationFunctionType

    e_pad = struct["e_pad"]
    nchk = struct["nchk"]
    ngroups = struct["ngroups"]
    calls_by_group = struct["calls_by_group"]
    entries = struct["entries"]
    entry_start = struct["entry_start"]
    rg = [list(range(NCORES))]

    nc = bacc.Bacc("TRN2", target_bir_lowering=False, debug=False,
                   num_devices=NCORES, num_swdge_queues=NQUEUES)

    xT_in = nc.dram_tensor("xT", [N_FEATS, NPC_PAD], f32, kind="ExternalInput")
    idx_in = nc.dram_tensor("idx", [128, e_pad // 16], dt.int16,
                            kind="ExternalInput")
    tloc_in = nc.dram_tensor("tloc", [128, nchk + TPC], f32,
                             kind="ExternalInput")
    dcol_in = nc.dram_tensor("dcol", [128, nchk + TPC], f32,
                             kind="ExternalInput")
    dinvo_in = nc.dram_tensor("dinvo", [128, TPC], f32, kind="ExternalInput")
    w0_in = nc.dram_tensor("w0", [N_FEATS, HIDDEN], f32, kind="ExternalInput")
    b0_in = nc.dram_tensor("b0", [HIDDEN, 1], f32, kind="ExternalInput")
    wp_in = nc.dram_tensor("wp", [HIDDEN, N_LAYERS * HIDDEN], bf16,
                           kind="ExternalInput")
    woute_in = nc.dram_tensor("woute", [HIDDEN + 1, N_CLASSES], bf16,
                              kind="ExternalInput")
    iota_in = nc.dram_tensor("iota", [128, 128], f32, kind="ExternalInput")
    identb_in = nc.dram_tensor("identb", [128, 128], bf16,
                               kind="ExternalInput")
    out_dram = nc.dram_tensor("out", [NPC_PAD, N_CLASSES], f32,
                              kind="ExternalOutput")

    with tile.TileContext(nc) as tc:
        with (
            tc.tile_pool(name="persist", bufs=1) as P,
            tc.tile_pool(name="work", bufs=3) as S,
            tc.tile_pool(name="msgp", bufs=4) as MSG,
            tc.tile_pool(name="ohp", bufs=8) as OH,
            tc.tile_pool(name="psagg", bufs=3, space="PSUM") as PSA,
            tc.tile_pool(name="psmisc", bufs=2, space="PSUM") as PSM,
            tc.tile_pool(name="pstp", bufs=2, space="PSUM") as PST,
            tc.tile_pool(name="pspers", bufs=1, space="PSUM") as PP,
            tc.tile_pool(name="dram", bufs=1, space="DRAM") as DR,
        ):
            def pload(name, src, shape, dtype=f32):
                t = P.tile(shape, dtype, tag=name)
                nc.sync.dma_start(t[:], src[:])
                return t

            iota_sb = pload("iota", iota_in, [128, 128])
            identb = pload("identb", identb_in, [128, 128], bf16)
            w0 = pload("w0", w0_in, [N_FEATS, HIDDEN])
            b0 = pload("b0", b0_in, [HIDDEN, 1])
            wp = pload("wp", wp_in, [HIDDEN, N_LAYERS * HIDDEN], bf16)
            woute = pload("woute", woute_in, [HIDDEN + 1, N_CLASSES], bf16)
            tloc = pload("tloc", tloc_in, [128, nchk + TPC])
            dcol = pload("dcol", dcol_in, [128, nchk + TPC])
            dinvo = pload("dinvo", dinvo_in, [128, TPC])
            h0s = P.tile([HIDDEN, NPC_PAD], bf16, tag="h0s")
            hs_own = P.tile([128, TPC * HIDDEN], bf16, tag="hs_own")
            # iota kept in PSUM so DVE one-hot builds read via the PSUM port
            # (1x mode, single SBUF write port): no DVE<->GpSimd SBUF-port
            # lock, so they overlap with SWDGE descriptor generation.
            iota_ps_full = PP.tile([128, 512], f32, tag="iops")
            iota_ps = iota_ps_full[:, 0:128]
            nc.vector.tensor_copy(iota_ps, iota_sb[:])
            # ACT one-hot operands: bf16 iota, negated tloc/dcol columns
            iotab = P.tile([128, 128], bf16, tag="iotab")
            nc.vector.tensor_copy(iotab[:], iota_sb[:])
            tlocn = P.tile([128, nchk + TPC], f32, tag="tlocn")
            nc.vector.tensor_scalar_mul(tlocn[:], tloc[:], -1.0)
            dcoln = P.tile([128, nchk + TPC], f32, tag="dcoln")
            nc.vector.tensor_scalar_mul(dcoln[:], dcol[:], -1.0)

            hs_shard = [
                DR.tile([NPC_PAD, 128], bf16, tag=f"shard{j}",
                        name=f"hs_shard{j}")
                for j in range(N_LAYERS)
            ]
            hs_full = [
                DR.tile([N_PAD, 128], bf16, tag=f"full{j}",
                        addr_space="Shared", name=f"hs_full{j}")
                for j in range(N_LAYERS)
            ]

            def emit_hs_out(src_ps, t, shard):
                """dinv-scale a node-major PSUM tile into hs_own and DMA
                both duplicated halves to the shard."""
                dst = hs_own[:, t * HIDDEN:(t + 1) * HIDDEN]
                nc.vector.tensor_scalar(dst, src_ps[:], dinvo[:, t:t + 1],
                                        None, op0=Alu.mult)
                nc.sync.dma_start(
                    shard[t * 128:(t + 1) * 128, 0:HIDDEN], dst)
                nc.sync.dma_start(
                    shard[t * 128:(t + 1) * 128, HIDDEN:128], dst)

            # ---------------- layer 0: h0 = relu(x @ W0 + b0) ----------------
            for t in range(TPC):
                xT = S.tile([N_FEATS, 128], f32, tag="xT")
                nc.sync.dma_start(xT[:], xT_in[:, t * 128:(t + 1) * 128])
                h_ps = PSM.tile([HIDDEN, 128], f32, tag="dense")
                nc.tensor.matmul(h_ps[:], lhsT=w0[:], rhs=xT[:],
                                 start=True, stop=True)
                h0t = S.tile([HIDDEN, 128], bf16, tag="h0t")
                nc.scalar.activation(h0t[:], h_ps[:], Act.Relu, bias=b0[:])
                nc.scalar.activation(
                    h0s[:, t * 128:(t + 1) * 128], h0t[:], Act.Copy,
                    scale=ALPHA)
                tp_ps = PST.tile([128, HIDDEN], bf16, tag="tpb")
                nc.tensor.transpose(tp_ps[:], h0t[:],
                                    identb[:HIDDEN, :HIDDEN])
                emit_hs_out(tp_ps, t, hs_shard[0])
            nc.gpsimd.collective_compute(
                "AllGather", Alu.bypass, replica_groups=rg,
                ins=[hs_shard[0].opt()], outs=[hs_full[0].opt()])

            # ---------------- GCNII layers ----------------
            call_no = 0
            oh_no = 0
            for li in range(N_LAYERS):
                cur = hs_full[li]
                is_last = li == N_LAYERS - 1
                nxt_shard = hs_shard[li + 1] if not is_last else None
                for g in range(ngroups):
                    tts = list(range(g * GROUP_TT,
                                     min((g + 1) * GROUP_TT, TPC)))
                    subs = [tts[i:i + SUB_TT]
                            for i in range(0, len(tts), SUB_TT)]
                    ps_tiles = [PSA.tile([HIDDEN, SUB_TT * 128], f32,
                                         tag="agg", name=f"agg{li}_{g}_{si}")
                                for si in range(len(subs))]

                    def onehot(j):
                        # alternate builders so one-hot production keeps up
                        # with SWDGE descriptor generation (DVE ~60%,
                        # ScalarE ~40% via relu(dcol*(1-(iota-tloc)^2)))
                        nonlocal oh_no
                        oh_no += 1
                        oh = OH.tile([128, 128], bf16, tag="oh")
                        if oh_no % 9 < 4:
                            tmp = OH.tile([128, 128], bf16, tag="ohtmp")
                            nc.scalar.activation(
                                tmp[:], iotab[:], Act.Square,
                                bias=tlocn[:, j:j + 1])
                            nc.scalar.activation(
                                oh[:], tmp[:], Act.Relu,
                                bias=dcol[:, j:j + 1],
                                scale=dcoln[:, j:j + 1])
                        else:
                            nc.vector.tensor_scalar(
                                oh[:], iota_ps[:], tloc[:, j:j + 1],
                                dcol[:, j:j + 1],
                                op0=Alu.is_equal, op1=Alu.mult)
                        return oh

                    for (kk, s, n) in calls_by_group[g]:
                        idx_t = S.tile([128, n // 16], dt.int16, tag="idxs")
                        nc.sync.dma_start(
                            idx_t[:], idx_in[:, s // 16:(s + n) // 16])
                        msg = MSG.tile([128, n // 128, 128], bf16,
                                       tag="msg")
                        rows_k = min(SRC_CHUNK, N_PAD - kk * SRC_CHUNK)
                        nc.gpsimd.dma_gather(
                            msg[:],
                            cur[kk * SRC_CHUNK:kk * SRC_CHUNK + rows_k, :],
                            idx_t[:], num_idxs=n, num_idxs_reg=n,
                            elem_size=128, single_packet=False,
                            queue_num=call_no % NQUEUES)
                        call_no += 1
                        for jj in range(n // 128):
                            j = s // 128 + jj
                            if not entries[j]:
                                continue
                            oh = onehot(j)
                            for ei, (t, lo, hi) in enumerate(entries[j]):
                                si = (t - g * GROUP_TT) // SUB_TT
                                ci = (t % SUB_TT) * 128
                                nc.tensor.matmul(
                                    ps_tiles[si][:, ci:ci + 128],
                                    lhsT=msg[lo:hi, jj, 0:HIDDEN],
                                    rhs=oh[lo:hi, :],
                                    start=bool(entry_start[j][ei]),
                                    stop=False,
                                    skip_group_check=True)
                    # diagonal (self-loop) term: one matmul per target tile
                    # reading the core's own node-major hs tile from SBUF
                    for t in tts:
                        oh = onehot(nchk + t)
                        si = (t - g * GROUP_TT) // SUB_TT
                        ci = (t % SUB_TT) * 128
                        nc.tensor.matmul(
                            ps_tiles[si][:, ci:ci + 128],
                            lhsT=hs_own[:, t * HIDDEN:(t + 1) * HIDDEN],
                            rhs=oh[:],
                            start=False, stop=(t % SUB_TT == SUB_TT - 1
                                               or t == tts[-1]),
                            skip_group_check=True)
                    for si, stts in enumerate(subs):
                        w = len(stts) * 128
                        n0 = stts[0] * 128
                        sup = S.tile([HIDDEN, SUB_TT * 128], bf16, tag="sup")
                        nc.vector.tensor_tensor(
                            out=sup[:, :w], in0=ps_tiles[si][:, :w],
                            in1=h0s[:, n0:n0 + w], op=Alu.add)
                        d_ps = PSM.tile([HIDDEN, SUB_TT * 128], f32,
                                        tag="dense")
                        nc.tensor.matmul(
                            d_ps[:, :w],
                            lhsT=wp[:, li * HIDDEN:(li + 1) * HIDDEN],
                            rhs=sup[:, :w], start=True, stop=True)
                        ht = S.tile([HIDDEN + 1, SUB_TT * 128], bf16,
                                    tag="ht")
                        nc.scalar.activation(ht[:HIDDEN, :w], d_ps[:, :w],
                                             Act.Relu)
                        if not is_last:
                            for tti, t in enumerate(stts):
                                tp_ps = PST.tile([128, HIDDEN], bf16,
                                                 tag="tpb")
                                nc.tensor.transpose(
                                    tp_ps[:],
                                    ht[:HIDDEN, tti * 128:(tti + 1) * 128],
                                    identb[:HIDDEN, :HIDDEN])
                                emit_hs_out(tp_ps, t, nxt_shard)
                        else:
                            # node-major output: out = ht^T @ [W_out; b_out]
                            # (bias folded as a ones row in ht)
                            nc.vector.memset(ht[HIDDEN:HIDDEN + 1, :w], 1.0)
                            for tti, t in enumerate(stts):
                                o_ps = PSM.tile([128, N_CLASSES], f32,
                                                tag="dense")
                                nc.tensor.matmul(
                                    o_ps[:],
                                    lhsT=ht[:, tti * 128:(tti + 1) * 128],
                                    rhs=woute[:],
                                    start=True, stop=True)
                                ot = S.tile([128, N_CLASSES], f32, tag="ot")
                                nc.vector.tensor_copy(ot[:], o_ps[:])
                                nc.sync.dma_start(
                                    out_dram[t * 128:(t + 1) * 128, :],
                                    ot[:])
                if not is_last:
                    nc.gpsimd.collective_compute(
                        "AllGather", Alu.bypass, replica_groups=rg,
                        ins=[nxt_shard.opt()],
                        outs=[hs_full[li + 1].opt()])

    nc.compile()
    return nc


def kernel(x, edge_index, W0, b0, Wl, W_out, b_out):
    from concourse.bass_utils import run_bass_kernel_spmd
    from concourse import mybir

    bf16np = mybir.dt.np(mybir.dt.bfloat16)

    x = np.asarray(x, dtype=np.float32)
    edge_index = np.asarray(edge_index)
    W0 = np.asarray(W0, dtype=np.float32)
    b0 = np.asarray(b0, dtype=np.float32)
    Wl = np.asarray(Wl, dtype=np.float32)
    W_out = np.asarray(W_out, dtype=np.float32)
    b_out = np.asarray(b_out, dtype=np.float32)

    struct, per_core = _preprocess(edge_index)
    nc = _build_program(struct)

    betas = np.array(
        [math.log(LAMDA / (i + 1) + 1.0) for i in range(N_LAYERS)],
        dtype=np.float32)
    wp_host = np.zeros((HIDDEN, N_LAYERS * HIDDEN), np.float32)
    eye = np.eye(HIDDEN, dtype=np.float32)
    for i in range(N_LAYERS):
        wp_host[:, i * HIDDEN:(i + 1) * HIDDEN] = (
            betas[i] * Wl[i] + (1.0 - betas[i]) * eye)
    woute_host = np.concatenate(
        [W_out, b_out.reshape(1, N_CLASSES)], axis=0)
    iota_host = np.tile(np.arange(128, dtype=np.float32), (128, 1))
    ident_host = np.eye(128, dtype=np.float32)

    in_maps = []
    for c in range(NCORES):
        xTc = np.zeros((N_FEATS, NPC_PAD), np.float32)
        xTc[:, :NPC] = x[c * NPC:(c + 1) * NPC].T
        pc = per_core[c]
        in_maps.append({
            "xT": xTc,
            "idx": pc["idx"],
            "tloc": pc["tloc"],
            "dcol": pc["dcol"],
            "dinvo": pc["dinvo"],
            "w0": W0,
            "b0": b0.reshape(HIDDEN, 1),
            "wp": wp_host.astype(bf16np),
            "woute": woute_host.astype(bf16np),
            "iota": iota_host,
            "identb": ident_host.astype(bf16np),
        })

    res = run_bass_kernel_spmd(
        nc, in_maps, core_ids=list(range(NCORES)), trace=TRACE)
    _LAST_RESULT["res"] = res
    out = np.concatenate(
        [res.results[c]["out"][:NPC] for c in range(NCORES)], axis=0)
    return out


# revision 25
# speedup vs baseline: 1.8013x; 1.0149x over previous
"""GCNII (nn_GCNII_17626545783193) Bass/Trainium2 kernel, 8 NeuronCores.

Strategy (target-node sharding, feature-major compute, bf16 aggregation):
  - Nodes sharded 12500/core (padded to 12544 = 98*128). Edges partitioned
    by target core. Self-loops are NOT gathered: they are folded in as one
    diagonal one-hot matmul per target tile reading the core's own
    node-major hs tile from SBUF.
  - Hidden states hs = dinv*h stored in bf16 with DUPLICATED halves:
    hs_full is [100352, 128] bf16 where row n = [h_n | h_n] (the dma_gather
    stride/elem floor is 256B). The matmul stationary slices cols 0:64.
  - gcn_norm factored: norm[e] = dinv[src]*dinv[dst]; dinv[src] folded into
    hs, 0.9*dinv[dst] folded into the one-hot scatter matrix (dcol).
  - Per layer, per core: dma_gather of in-edge source rows from hs_full
    across 4 SWDGE queues (overlapped drains; ~5.4ns/idx); DVE builds bf16
    one-hot [128edge, 128target] tiles (iota==tloc)*dcol reading iota from
    PSUM (1-port mode - avoids the DVE<->GpSimd shared-SBUF-port lock that
    would serialize one-hot builds against SWDGE descriptor generation);
    PE computes agg^T = msgs^T @ onehot (bf16, 1 cyc/row) accumulated in
    PSUM [64, 512]; support = agg + 0.1*h0^T (DVE); GCNII dense update via
    ONE matmul with host-fused W' = beta*Wl + (1-beta)*I; ACT relu; PE
    transpose back to node-major; scale by dinv into a persistent SBUF
    hs_own tile (feeds next layer's diagonal term); DMA both 64-col halves
    to the shard; AllGather.
"""
import sys
sys.path.insert(0, "/opt/trn_rl_repo")

import math
import numpy as np

# ---- problem constants (hardcoded per spec) ----
N_NODES = 100000
N_FEATS = 128
HIDDEN = 64
N_CLASSES = 40
N_LAYERS = 8
ALPHA = 0.1
LAMDA = 0.5
NCORES = 8

NPC = N_NODES // NCORES            # 12500 real nodes per core
TPC = (NPC + 127) // 128           # 98 target tiles per core
NPC_PAD = TPC * 128                # 12544
N_PAD = NPC_PAD * NCORES           # 100352
GROUP_TT = 8                       # target tiles per gather group
SUB_TT = 4                         # target tiles per psum tile (512 cols)
SRC_CHUNK = 32768                  # int16 index range per gather source chunk
NCHUNKS = (N_PAD + SRC_CHUNK - 1) // SRC_CHUNK  # 4
CALL_MAX = 8192                    # max indices per dma_gather call
NQUEUES = 4                        # SWDGE queues (overlapped gather drains)

TRACE = False          # test.py sets this for profiling
_LAST_RESULT = {}      # test.py reads exec_time from here


def _preprocess(edge_index):
    """Build the uniform (cross-core identical) edge stream structure and the
    per-core data arrays.  Self-loops are excluded from the gather stream
    (handled as diagonal matmuls on-chip)."""
    row = np.asarray(edge_index[0], dtype=np.int64)
    col = np.asarray(edge_index[1], dtype=np.int64)
    deg = np.bincount(col, minlength=N_NODES).astype(np.float32) + 1.0
    dinv = (1.0 / np.sqrt(deg)).astype(np.float32)

    r = row
    cl = col
    core = cl // NPC
    lcol = cl - core * NPC                       # 0..12499 local target
    tt = lcol >> 7                               # target tile 0..97
    gp = (r // NPC) * NPC_PAD + (r % NPC)        # padded global source id
    k = gp // SRC_CHUNK                          # source chunk 0..3

    ngroups = (TPC + GROUP_TT - 1) // GROUP_TT

    slot = (core * TPC + tt) * NCHUNKS + k
    counts = np.bincount(slot, minlength=NCORES * TPC * NCHUNKS).reshape(
        NCORES, TPC, NCHUNKS
    )
    maxc = counts.max(axis=0)                    # [TPC, NCHUNKS]
    # 128-granularity slots (one full matmul per chunk; sliced-partition
    # half-chunk matmuls crash the hardware, so no 64-grain packing)
    slot_len = (-(-maxc // 128) * 128).astype(np.int64)

    slot_off = np.zeros((TPC, NCHUNKS), np.int64)
    off = 0
    calls_by_group = [[] for _ in range(ngroups)]
    for g in range(ngroups):
        tts = range(g * GROUP_TT, min((g + 1) * GROUP_TT, TPC))
        for kk in range(NCHUNKS):
            run_start = off
            for t in tts:
                slot_off[t, kk] = off
                off += slot_len[t, kk]
            if (off - run_start) % 128:
                off += 64            # keep calls 128-aligned
            s = run_start
            while s < off:
                n = min(CALL_MAX, off - s)
                calls_by_group[g].append((kk, int(s), int(n)))
                s += n
    e_pad = off
    nchk = e_pad // 128
    # chunk entries: per 128-row chunk, the (tile, row_lo, row_hi) spans of
    # the slots overlapping it (1 normally, 2 when straddling).
    entries = [[] for _ in range(nchk)]
    for t in range(TPC):
        for kk in range(NCHUNKS):
            s0 = int(slot_off[t, kk])
            s1 = s0 + int(slot_len[t, kk])
            j = s0 // 128
            while j * 128 < s1:
                lo = max(s0, j * 128)
                hi = min(s1, (j + 1) * 128)
                entries[j].append((t, lo - j * 128, hi - j * 128))
                j += 1
    # start flags: per PSUM tile (= subgroup of SUB_TT ttiles): exactly one
    # start=True (chronologically first matmul into the bank). The diagonal
    # (self-loop) matmul is appended after the group's gather chunks and is
    # always chronologically last for its sub, so it carries stop=True.
    seen_sub = set()
    entry_start = []
    for j in range(nchk):
        es = []
        for (t, lo, hi) in entries[j]:
            sg = t // SUB_TT
            es.append(sg not in seen_sub)
            seen_sub.add(sg)
        entry_start.append(es)

    per_core = []
    for c in range(NCORES):
        m = core == c
        clm = cl[m]
        lc = lcol[m]
        tc_ = tt[m]
        kc = k[m]
        gpc = gp[m]
        # sort by slot, then ascending source id within the slot (possible
        # HBM locality for the gather engines)
        skey = tc_ * NCHUNKS + kc
        order = np.lexsort((gpc, skey))
        sorted_key = skey[order]
        firsts = np.searchsorted(sorted_key, sorted_key, side="left")
        ranks = np.arange(len(sorted_key)) - firsts
        pos = slot_off[tc_[order], kc[order]] + ranks

        idx_stream = np.zeros(e_pad, np.int16)
        tloc_stream = np.zeros(e_pad, np.float32)
        dcol_stream = np.zeros(e_pad, np.float32)
        idx_stream[pos] = (gpc[order] - kc[order] * SRC_CHUNK).astype(np.int16)
        tloc_stream[pos] = (lc[order] & 127).astype(np.float32)
        dcol_stream[pos] = ((1.0 - ALPHA) * dinv[clm[order]]).astype(np.float32)

        idx_w = np.zeros((128, e_pad // 16), np.int16)
        for g in range(ngroups):
            for (kk, s, n) in calls_by_group[g]:
                blk = idx_stream[s:s + n].reshape(n // 16, 16).T
                idx_w[:, s // 16:(s + n) // 16] = np.tile(blk, (8, 1))
        # diag (self-loop) columns appended at nchk + t: tloc = partition id,
        # dcol = 0.9 * dinv^2 of the tile's local nodes
        dvl = np.zeros(NPC_PAD, np.float32)
        dvl[:NPC] = dinv[c * NPC:(c + 1) * NPC]
        diag_tloc = np.tile(np.arange(128, dtype=np.float32), (TPC, 1))
        # hs_own rows already carry one dinv factor; dcol supplies the other
        diag_dcol = (1.0 - ALPHA) * dvl.reshape(TPC, 128)
        tloc_arr = np.ascontiguousarray(
            np.concatenate([tloc_stream.reshape(nchk, 128), diag_tloc]).T)
        dcol_arr = np.ascontiguousarray(
            np.concatenate([dcol_stream.reshape(nchk, 128), diag_dcol]).T)
        dinvo = np.ascontiguousarray(dvl.reshape(TPC, 128).T)
        per_core.append(
            dict(idx=idx_w, tloc=tloc_arr, dcol=dcol_arr, dinvo=dinvo)
        )

    struct = dict(
        e_pad=e_pad,
        nchk=nchk,
        ngroups=ngroups,
        calls_by_group=calls_by_group,
        entries=entries,
        entry_start=entry_start,
    )
    return struct, per_core


def _build_program(struct):
    import concourse.bass as bass
    import concourse.mybir as mybir
    import concourse.tile as tile
    import concourse.bacc as bacc

    dt = mybir.dt
    f32 = dt.float32
    bf16 = dt.bfloat16
    Alu = mybir.AluOpType
    Act = mybir.ActivationFunctionType

    e_pad = struct["e_pad"]
    nchk = struct["nchk"]
    ngroups = struct["ngroups"]
    calls_by_group = struct["calls_by_group"]
    entries = struct["entries"]
    entry_start = struct["entry_start"]
    rg = [list(range(NCORES))]

    nc = bacc.Bacc("TRN2", target_bir_lowering=False, debug=False,
                   num_devices=NCORES, num_swdge_queues=NQUEUES)

    xT_in = nc.dram_tensor("xT", [N_FEATS, NPC_PAD], f32, kind="ExternalInput")
    idx_in = nc.dram_tensor("idx", [128, e_pad // 16], dt.int16,
                            kind="ExternalInput")
    tloc_in = nc.dram_tensor("tloc", [128, nchk + TPC], f32,
                             kind="ExternalInput")
    dcol_in = nc.dram_tensor("dcol", [128, nchk + TPC], f32,
                             kind="ExternalInput")
    dinvo_in = nc.dram_tensor("dinvo", [128, TPC], f32, kind="ExternalInput")
    w0_in = nc.dram_tensor("w0", [N_FEATS, HIDDEN], f32, kind="ExternalInput")
    b0_in = nc.dram_tensor("b0", [HIDDEN, 1], f32, kind="ExternalInput")
    wp_in = nc.dram_tensor("wp", [HIDDEN, N_LAYERS * HIDDEN], bf16,
                           kind="ExternalInput")
    woute_in = nc.dram_tensor("woute", [HIDDEN + 1, N_CLASSES], bf16,
                              kind="ExternalInput")
    iota_in = nc.dram_tensor("iota", [128, 128], f32, kind="ExternalInput")
    identb_in = nc.dram_tensor("identb", [128, 128], bf16,
                               kind="ExternalInput")
    out_dram = nc.dram_tensor("out", [NPC_PAD, N_CLASSES], f32,
                              kind="ExternalOutput")

    with tile.TileContext(nc) as tc:
        with (
            tc.tile_pool(name="persist", bufs=1) as P,
            tc.tile_pool(name="work", bufs=3) as S,
            tc.tile_pool(name="msgp", bufs=6) as MSG,
            tc.tile_pool(name="ohp", bufs=16) as OH,
            tc.tile_pool(name="psagg", bufs=4, space="PSUM") as PSA,
            tc.tile_pool(name="psmisc", bufs=2, space="PSUM") as PSM,
            tc.tile_pool(name="pstp", bufs=1, space="PSUM") as PST,
            tc.tile_pool(name="pspers", bufs=1, space="PSUM") as PP,
            tc.tile_pool(name="dram", bufs=1, space="DRAM") as DR,
        ):
            def pload(name, src, shape, dtype=f32):
                t = P.tile(shape, dtype, tag=name)
                nc.sync.dma_start(t[:], src[:])
                return t

            iota_sb = pload("iota", iota_in, [128, 128])
            identb = pload("identb", identb_in, [128, 128], bf16)
            w0 = pload("w0", w0_in, [N_FEATS, HIDDEN])
            b0 = pload("b0", b0_in, [HIDDEN, 1])
            wp = pload("wp", wp_in, [HIDDEN, N_LAYERS * HIDDEN], bf16)
            woute = pload("woute", woute_in, [HIDDEN + 1, N_CLASSES], bf16)
            tloc = pload("tloc", tloc_in, [128, nchk + TPC])
            dcol = pload("dcol", dcol_in, [128, nchk + TPC])
            dinvo = pload("dinvo", dinvo_in, [128, TPC])
            h0s = P.tile([HIDDEN, NPC_PAD], bf16, tag="h0s")
            hs_own = P.tile([128, TPC * HIDDEN], bf16, tag="hs_own")
            # iota kept in PSUM so DVE one-hot builds read via the PSUM port
            # (1x mode, single SBUF write port): no DVE<->GpSimd SBUF-port
            # lock, so they overlap with SWDGE descriptor generation.
            iota_ps_full = PP.tile([128, 512], f32, tag="iops")
            iota_ps = iota_ps_full[:, 0:128]
            nc.vector.tensor_copy(iota_ps, iota_sb[:])
            # ACT one-hot operands: bf16 iota, negated tloc/dcol columns
            iotab = P.tile([128, 128], bf16, tag="iotab")
            nc.vector.tensor_copy(iotab[:], iota_sb[:])
            tlocn = P.tile([128, nchk + TPC], f32, tag="tlocn")
            nc.vector.tensor_scalar_mul(tlocn[:], tloc[:], -1.0)
            dcoln = P.tile([128, nchk + TPC], f32, tag="dcoln")
            nc.vector.tensor_scalar_mul(dcoln[:], dcol[:], -1.0)

            hs_shard = [
                DR.tile([NPC_PAD, 128], bf16, tag=f"shard{j}",
                        name=f"hs_shard{j}")
                for j in range(N_LAYERS)
            ]
            hs_full = [
                DR.tile([N_PAD, 128], bf16, tag=f"full{j}",
                        addr_space="Shared", name=f"hs_full{j}")
                for j in range(N_LAYERS)
            ]

            def emit_hs_out(src_ps, t, shard):
                """dinv-scale a node-major PSUM tile into hs_own and DMA
                both duplicated halves to the shard."""
                dst = hs_own[:, t * HIDDEN:(t + 1) * HIDDEN]
                nc.vector.tensor_scalar(dst, src_ps[:], dinvo[:, t:t + 1],
                                        None, op0=Alu.mult)
                nc.sync.dma_start(
                    shard[t * 128:(t + 1) * 128, 0:HIDDEN], dst)
                nc.sync.dma_start(
                    shard[t * 128:(t + 1) * 128, HIDDEN:128], dst)

            # ---------------- layer 0: h0 = relu(x @ W0 + b0) ----------------
            for t in range(TPC):
                xT = S.tile([N_FEATS, 128], f32, tag="xT")
                nc.sync.dma_start(xT[:], xT_in[:, t * 128:(t + 1) * 128])
                h_ps = PSM.tile([HIDDEN, 128], f32, tag="dense")
                nc.tensor.matmul(h_ps[:], lhsT=w0[:], rhs=xT[:],
                                 start=True, stop=True)
                h0t = S.tile([HIDDEN, 128], bf16, tag="h0t")
                nc.scalar.activation(h0t[:], h_ps[:], Act.Relu, bias=b0[:])
                nc.scalar.activation(
                    h0s[:, t * 128:(t + 1) * 128], h0t[:], Act.Copy,
                    scale=ALPHA)
                tp_ps = PST.tile([128, HIDDEN], bf16, tag="tpb")
                nc.tensor.transpose(tp_ps[:], h0t[:],
                                    identb[:HIDDEN, :HIDDEN])
                emit_hs_out(tp_ps, t, hs_shard[0])
            nc.gpsimd.collective_compute(
                "AllGather", Alu.bypass, replica_groups=rg,
                ins=[hs_shard[0].opt()], outs=[hs_full[0].opt()])

            # ---------------- GCNII layers ----------------
            call_no = 0
            oh_no = 0
            for li in range(N_LAYERS):
                cur = hs_full[li]
                is_last = li == N_LAYERS - 1
                nxt_shard = hs_shard[li + 1] if not is_last else None
                for g in range(ngroups):
                    tts = list(range(g * GROUP_TT,
                                     min((g + 1) * GROUP_TT, TPC)))
                    subs = [tts[i:i + SUB_TT]
                            for i in range(0, len(tts), SUB_TT)]
                    ps_tiles = [PSA.tile([HIDDEN, SUB_TT * 128], f32,
                                         tag="agg", name=f"agg{li}_{g}_{si}")
                                for si in range(len(subs))]

                    def onehot(j):
                        # alternate builders so one-hot production keeps up
                        # with SWDGE descriptor generation (DVE ~60%,
                        # ScalarE ~40% via relu(dcol*(1-(iota-tloc)^2)))
                        nonlocal oh_no
                        oh_no += 1
                        oh = OH.tile([128, 128], bf16, tag="oh")
                        if oh_no % 3 == 0:
                            tmp = OH.tile([128, 128], bf16, tag="ohtmp")
                            nc.scalar.activation(
                                tmp[:], iotab[:], Act.Square,
                                bias=tlocn[:, j:j + 1])
                            nc.scalar.activation(
                                oh[:], tmp[:], Act.Relu,
                                bias=dcol[:, j:j + 1],
                                scale=dcoln[:, j:j + 1])
                        else:
                            nc.vector.tensor_scalar(
                                oh[:], iota_ps[:], tloc[:, j:j + 1],
                                dcol[:, j:j + 1],
                                op0=Alu.is_equal, op1=Alu.mult)
                        return oh

                    for (kk, s, n) in calls_by_group[g]:
                        idx_t = S.tile([128, n // 16], dt.int16, tag="idxs")
                        nc.sync.dma_start(
                            idx_t[:], idx_in[:, s // 16:(s + n) // 16])
                        msg = MSG.tile([128, n // 128, 128], bf16,
                                       tag="msg")
                        rows_k = min(SRC_CHUNK, N_PAD - kk * SRC_CHUNK)
                        nc.gpsimd.dma_gather(
                            msg[:],
                            cur[kk * SRC_CHUNK:kk * SRC_CHUNK + rows_k, :],
                            idx_t[:], num_idxs=n, num_idxs_reg=n,
                            elem_size=128, single_packet=False,
                            queue_num=call_no % NQUEUES)
                        call_no += 1
                        for jj in range(n // 128):
                            j = s // 128 + jj
                            if not entries[j]:
                                continue
                            oh = onehot(j)
                            for ei, (t, lo, hi) in enumerate(entries[j]):
                                si = (t - g * GROUP_TT) // SUB_TT
                                ci = (t % SUB_TT) * 128
                                nc.tensor.matmul(
                                    ps_tiles[si][:, ci:ci + 128],
                                    lhsT=msg[lo:hi, jj, 0:HIDDEN],
                                    rhs=oh[lo:hi, :],
                                    start=bool(entry_start[j][ei]),
                                    stop=False,
                                    skip_group_check=True)
                    # diagonal (self-loop) term: one matmul per target tile
                    # reading the core's own node-major hs tile from SBUF
                    for t in tts:
                        oh = onehot(nchk + t)
                        si = (t - g * GROUP_TT) // SUB_TT
                        ci = (t % SUB_TT) * 128
                        nc.tensor.matmul(
                            ps_tiles[si][:, ci:ci + 128],
                            lhsT=hs_own[:, t * HIDDEN:(t + 1) * HIDDEN],
                            rhs=oh[:],
                            start=False, stop=(t % SUB_TT == SUB_TT - 1
                                               or t == tts[-1]),
                            skip_group_check=True)
                    for si, stts in enumerate(subs):
                        w = len(stts) * 128
                        n0 = stts[0] * 128
                        sup = S.tile([HIDDEN, SUB_TT * 128], bf16, tag="sup")
                        nc.vector.tensor_tensor(
                            out=sup[:, :w], in0=ps_tiles[si][:, :w],
                            in1=h0s[:, n0:n0 + w], op=Alu.add)
                        d_ps = PSM.tile([HIDDEN, SUB_TT * 128], f32,
                                        tag="dense")
                        nc.tensor.matmul(
                            d_ps[:, :w],
                            lhsT=wp[:, li * HIDDEN:(li + 1) * HIDDEN],
                            rhs=sup[:, :w], start=True, stop=True)
                        ht = S.tile([HIDDEN + 1, SUB_TT * 128], bf16,
                                    tag="ht")
                        nc.scalar.activation(ht[:HIDDEN, :w], d_ps[:, :w],
                                             Act.Relu)
                        if not is_last:
                            for tti, t in enumerate(stts):
                                tp_ps = PST.tile([128, HIDDEN], bf16,
                                                 tag="tpb")
                                nc.tensor.transpose(
                                    tp_ps[:],
                                    ht[:HIDDEN, tti * 128:(tti + 1) * 128],
                                    identb[:HIDDEN, :HIDDEN])
                                emit_hs_out(tp_ps, t, nxt_shard)
                        else:
                            # node-major output: out = ht^T @ [W_out; b_out]
                            # (bias folded as a ones row in ht)
                            nc.vector.memset(ht[HIDDEN:HIDDEN + 1, :w], 1.0)
                            for tti, t in enumerate(stts):
                                o_ps = PSM.tile([128, N_CLASSES], f32,
                                                tag="dense")
                                nc.tensor.matmul(
                                    o_ps[:],
                                    lhsT=ht[:, tti * 128:(tti + 1) * 128],
                                    rhs=woute[:],
                                    start=True, stop=True)
                                ot = S.tile([128, N_CLASSES], f32, tag="ot")
                                nc.vector.tensor_copy(ot[:], o_ps[:])
                                nc.sync.dma_start(
                                    out_dram[t * 128:(t + 1) * 128, :],
                                    ot[:])
                if not is_last:
                    nc.gpsimd.collective_compute(
                        "AllGather", Alu.bypass, replica_groups=rg,
                        ins=[nxt_shard.opt()],
                        outs=[hs_full[li + 1].opt()])

    nc.compile()
    return nc


def kernel(x, edge_index, W0, b0, Wl, W_out, b_out):
    from concourse.bass_utils import run_bass_kernel_spmd
    from concourse import mybir

    bf16np = mybir.dt.np(mybir.dt.bfloat16)

    x = np.asarray(x, dtype=np.float32)
    edge_index = np.asarray(edge_index)
    W0 = np.asarray(W0, dtype=np.float32)
    b0 = np.asarray(b0, dtype=np.float32)
    Wl = np.asarray(Wl, dtype=np.float32)
    W_out = np.asarray(W_out, dtype=np.float32)
    b_out = np.asarray(b_out, dtype=np.float32)

    struct, per_core = _preprocess(edge_index)
    nc = _build_program(struct)

    betas = np.array(
        [math.log(LAMDA / (i + 1) + 1.0) for i in range(N_LAYERS)],
        dtype=np.float32)
    wp_host = np.zeros((HIDDEN, N_LAYERS * HIDDEN), np.float32)
    eye = np.eye(HIDDEN, dtype=np.float32)
    for i in range(N_LAYERS):
        wp_host[:, i * HIDDEN:(i + 1) * HIDDEN] = (
            betas[i] * Wl[i] + (1.0 - betas[i]) * eye)
    woute_host = np.concatenate(
        [W_out, b_out.reshape(1, N_CLASSES)], axis=0)
    iota_host = np.tile(np.arange(128, dtype=np.float32), (128, 1))
    ident_host = np.eye(128, dtype=np.float32)

    in_maps = []
    for c in range(NCORES):
        xTc = np.zeros((N_FEATS, NPC_PAD), np.float32)
        xTc[:, :NPC] = x[c * NPC:(c + 1) * NPC].T
        pc = per_core[c]
        in_maps.append({
            "xT": xTc,
            "idx": pc["idx"],
            "tloc": pc["tloc"],
            "dcol": pc["dcol"],
            "dinvo": pc["dinvo"],
            "w0": W0,
            "b0": b0.reshape(HIDDEN, 1),
            "wp": wp_host.astype(bf16np),
            "woute": woute_host.astype(bf16np),
            "iota": iota_host,
            "identb": ident_host.astype(bf16np),
        })

    res = run_bass_kernel_spmd(
        nc, in_maps, core_ids=list(range(NCORES)), trace=TRACE)
    _LAST_RESULT["res"] = res
    out = np.concatenate(
        [res.results[c]["out"][:NPC] for c in range(NCORES)], axis=0)
    return out
